# revision 7
# baseline (speedup 1.0000x reference)
"""Trainium2 Bass kernel for the MACE-style symmetric contraction:

    out  = einsum("xyik,kc,bci->bcxy", U3, w3, nf)
    c2   = einsum("xyk,kc->cxy", U2, w2)[None] + out
    out  = einsum("bcxi,bci->bcx", c2, nf)
    c1   = einsum("xk,kc->cx", U1, w1)[None] + out
    out  = einsum("bci,bci->bc", c1, nf)

Algebraically:

    out[b,c] =   sum_{x,y,i} W3U[x,y,i,c] nf[b,c,x] nf[b,c,y] nf[b,c,i]
               + sum_{x,y}   U2w2[c,x,y]  nf[b,c,x] nf[b,c,y]
               + sum_{x}     U1w1[c,x]    nf[b,c,x]

with W3U = einsum("xyik,kc->xyic", U3, w3).  Since nf_x*nf_y is symmetric
in (x,y), only the (x,y)-symmetric part of W3U/U2w2 contributes: fold the
rectangle onto unordered pairs {X, y<=X} via SYM[X,y] = W3U[X,y]+W3U[y,X]
(diagonal halved).  This halves the U3 HBM stream, the build matmul
columns, and the phase-B work vs the unfolded form.  The U1 term is added
on the host (tiny).

Sharding: each core owns 6 X-values {r, 15-r, 16+r, 31-r, 32+r, 47-r},
paired into 3 fold groups (Xa, Xb=47-Xa).  A group's 49 columns are
[Xa-run: y=0..Xa][Xb-run: y=0..Xb] - rectangular across cores, so one
SPMD program serves all cores; per-core structure lives in the data.

Device pipeline (fp16 data, fp32 PSUM/scan state):
  build:   A2[c, i', (g,w)] = w3.T @ u3s on PE, k-accumulated in PSUM,
           drained fp16 to a DRAM scratch (i'=48 row carries folded U2w2,
           contracted against a ones channel in nfa).  Scratch writes go
           out on the (otherwise idle) gpsimd SWDGE queue so they never
           block the sync queue's u3 stream.
  phase B: per (4-c block, b-tile): Z[b,(g,w)] = nfa.T @ A2_c on PE into
           a [128,2048] PSUM tile (4 c's); one fused DVE MAC-scan against
           the host-streamed weight tensor nfprod[b,c,w] = nf_y(w)*nf_X(w)
           accumulates Z*nfprod, so the scan value at the end of each c's
           147 columns IS out[b,c] (recovered by a shifted subtract).
"""

import numpy as np

B = 512          # atoms
C = 96           # feats
I = 48           # irreps
K3, K2, K1 = 1270, 24, 3
NCORES = 8
I1 = I + 1        # 49 contraction rows (i + U2 aug row)
W = 49            # folded group width
G = 3             # fold groups per core
NW = G * W        # 147 columns per core
MP = I * NW       # 7056 build m-columns (m = i*NW + g*W + w)
SCR = I1 * NW     # 7203 scratch cols per c (aug row at 7056..7202)
KP = 1280         # K3 padded to 10 partition tiles
KT = KP // 128    # 10
MCHUNK = 2048     # build chunk (one PSUM tile, 4 banks)
PAIRS = C // 2    # 48
NT = B // 128     # 4 b-tiles
NQ = C // 4       # 24 phase-B units per b-tile (4 c's each)

_CACHE = {}

# exec time of the last device run (ns), when BASS_TRACE=1
LAST_EXEC_NS = None


def _core_pairs(r):
    """Fold pairs (Xa, Xb) with Xa+Xb=47; Xa-run first (y=0..Xa)."""
    return [(r, 47 - r), (15 - r, 32 + r), (16 + r, 31 - r)]


def _register_mac_scan():
    """Custom DVE op: out[t] = prefix-sum of in0[t]*in1[t] (fp32 state)."""
    import concourse.dve_ops as dve_ops_mod
    if any(op.name == "TT_MAC_SCAN_ANT" for op in dve_ops_mod.OPS):
        return next(op for op in dve_ops_mod.OPS
                    if op.name == "TT_MAC_SCAN_ANT")
    from concourse.dve_spec import Spec, scan, Src0, Src1
    from concourse.dve_uop import AluOp
    from concourse.dve_ops import DveOp

    def _ref_mac_scan(in0, in1, s0, s1, imm2):
        p = in0.astype(np.float32) * in1.astype(np.float32)
        return np.cumsum(p.reshape(p.shape[0], -1), axis=1).reshape(
            p.shape).astype(np.float32)

    spec = Spec(body=scan(AluOp.ADD, Src0 * Src1), reference=_ref_mac_scan)
    op = DveOp("TT_MAC_SCAN_ANT", spec, subdim=False,
               uops_sha={"v3": "b3fc3e78a862b7eb",
                         "v4": "bc6a002865d48b97"})
    dve_ops_mod.OPS.append(op)
    dve_ops_mod.CUSTOM_DVE_SPECS[op.name] = spec
    dve_ops_mod._SUB_OPCODE_FOR_NAME[op.name] = (
        max(dve_ops_mod._SUB_OPCODE_FOR_NAME.values()) + 1)
    return op


def _build_nc(debug=None):
    import concourse.mybir as mybir
    from concourse.tile import TileContext

    mac_scan = _register_mac_scan()

    f16 = mybir.dt.float16
    f32 = mybir.dt.float32
    sub = mybir.AluOpType.subtract

    import concourse.bacc as bacc
    nc = bacc.Bacc(None, target_bir_lowering=False)
    u3t = nc.dram_tensor("u3t", [KP, MP], f16, kind="ExternalInput")
    # w3p pre-swizzled host-side to [p, kt*C] so the load is contiguous;
    # its c-axis is reordered [even c's | odd c's] so the build PSUM rows
    # land even-c in rows 0..47, odd-c in 48..95.
    w3p = nc.dram_tensor("w3p", [128, KT * C], f16, kind="ExternalInput")
    # nfa[p, t, cp, b128]: p = 64*(c%2) + i'; i'=48 row is ones
    nfa = nc.dram_tensor("nfa", [128, NT * PAIRS * 128], f16,
                         kind="ExternalInput")
    # nfprod[b, c*NW]: per-column weight nf_y(w)*nf_X(w)
    nfprod = nc.dram_tensor("nfprod", [B, C * NW], f16,
                            kind="ExternalInput")
    u2aug = nc.dram_tensor("u2aug", [32, NW], f16, kind="ExternalInput")
    w21 = nc.dram_tensor("w21", [32, C], f16, kind="ExternalInput")
    outp = nc.dram_tensor("out", [B, C], f32, kind="ExternalOutput")

    with TileContext(nc) as tc:
        with (
            nc.allow_low_precision(reason="fp16 intermediates; rel-err "
                                   "budget 2e-2 vs ~1e-3 incurred"),
            tc.tile_pool(name="dram", bufs=1, space="DRAM") as dpool,
            tc.tile_pool(name="const", bufs=1) as cpool,
            tc.tile_pool(name="u3", bufs=6) as u3pool,
            tc.tile_pool(name="psum", bufs=2, space="PSUM") as psum,
            tc.tile_pool(name="sc", bufs=2) as scpool,
            tc.tile_pool(name="nfr", bufs=2) as nfrpool,
            tc.tile_pool(name="nfat", bufs=2) as nfapool,
            tc.tile_pool(name="stg", bufs=3) as stgpool,
            tc.tile_pool(name="fin", bufs=2) as finpool,
        ):
            # scratch row c = [i-major build cols 0..7055 | aug 7056..7202]
            # rows: even c's at 0..47, odd at 48..95 (w3p reorder)
            w3u_scr = dpool.tile([C, SCR], f16)

            # ---- u3 stream + build own the sync queue; residents go on
            # the scalar queue; scratch writes on the gpsimd queue ----
            w3sb = cpool.tile([128, KT * C], f16)
            nc.sync.dma_start(out=w3sb[:, :], in_=w3p[:, :])
            w3v = w3sb[:, :].rearrange("p (k c) -> p k c", c=C)
            w21sb = cpool.tile([32, C], f16)
            nc.sync.dma_start(out=w21sb[:, :], in_=w21[:, :])
            u2sb = cpool.tile([32, NW], f16)
            nc.sync.dma_start(out=u2sb[:, :], in_=u2aug[:, :])

            nfav = nfa[:, :].rearrange("p (t m) -> p t m", t=NT)

            def load_nfa(t):
                nt = nfapool.tile([128, PAIRS * 128], f16, tag="nfa")
                for par in (0, 1):
                    r0 = 64 * par
                    nc.scalar.dma_start(out=nt[r0:r0 + I1, :],
                                        in_=nfav[r0:r0 + I1, t, :])
                return nt

            def load_nfprod(t):
                nt = nfrpool.tile([128, C * NW], f16, tag="nfr")
                nc.scalar.dma_start(out=nt[:, :],
                                    in_=nfprod[t * 128:(t + 1) * 128, :])
                return nt

            nfa_t = load_nfa(0)
            nfp_t = load_nfprod(0)

            # lt: one resident tile; loaded piecewise as the scratch
            # completes (even-c c0..c47 to rows 0..48, odd-c to 64..112)
            lt = cpool.tile([128, PAIRS * NW], f16, name="ltbig")
            ltv = lt[:, :].rearrange("p (c m) -> p c m", m=NW)
            scr3 = w3u_scr[:, :].rearrange("c (i m) -> c i m", m=NW)

            def load_lt_rows(i_lo, i_hi):
                # gpsimd queue: FIFO-ordered behind the scratch write that
                # produced these rows, so no extra sync is needed.
                for par in (0, 1):
                    nc.gpsimd.dma_start(
                        out=ltv[64 * par + i_lo:64 * par + i_hi, :, :],
                        in_=scr3[48 * par:48 * par + 48,
                                 i_lo:i_hi, :].rearrange(
                                     "c i m -> i c m"))

            # ---- aug build: [96, 147] = w21.T @ u2aug -> aug row ----
            aps = psum.tile([128, MCHUNK], f32, tag="z")
            nc.tensor.matmul(aps[:C, :NW], w21sb[:K2, :], u2sb[:K2, :],
                             start=True, stop=True)
            astg = stgpool.tile([C, MCHUNK], f16, tag="stg")
            nc.scalar.copy(astg[:, :NW], aps[:C, :NW])
            nc.gpsimd.dma_start(out=w3u_scr[:, MP:SCR], in_=astg[:, :NW])
            load_lt_rows(I, I1)  # aug row

            # ---- build: A2[c, m] = w3.T @ u3s, k-accumulated; u3 tiles
            # alternate sync/scalar queues to hide per-DMA gaps ----
            NMC = (MP + MCHUNK - 1) // MCHUNK  # 4 (2048,2048,2048,912)
            i_done = 0
            for mcp in range(NMC):
                wc = min(MCHUNK, MP - mcp * MCHUNK)
                ps = psum.tile([128, MCHUNK], f32, tag="z",
                               name=f"bp{mcp}")
                for kt in range(KT):
                    tl = u3pool.tile([128, MCHUNK], f16, tag="u3")
                    base = mcp * MCHUNK
                    eng = nc.sync if kt % 2 == 0 else nc.scalar
                    eng.dma_start(
                        out=tl[:, :wc],
                        in_=u3t[kt * 128:(kt + 1) * 128, base:base + wc])
                    for off in range(0, wc, 512):
                        h = min(512, wc - off)
                        nc.tensor.matmul(
                            ps[:C, off:off + h], w3v[:, kt, :],
                            tl[:, off:off + h],
                            start=(kt == 0), stop=(kt == KT - 1))
                stg = stgpool.tile([C, MCHUNK], f16, tag="stg")
                nc.scalar.copy(stg[:, :wc], ps[:C, :wc])
                nc.gpsimd.dma_start(
                    out=w3u_scr[:, mcp * MCHUNK:mcp * MCHUNK + wc],
                    in_=stg[:, :wc])
                i_new = ((mcp + 1) * MCHUNK) // NW if mcp + 1 < NMC else I
                if i_new > i_done:
                    load_lt_rows(i_done, i_new)
                    i_done = i_new

            # ---- phase B, t-major; unit = 4 consecutive c's ----
            nq = NQ if debug is None else debug
            for t in range(NT):
                if t + 1 < NT:
                    nfa_next = load_nfa(t + 1)
                    nfp_next = load_nfprod(t + 1)
                sct = scpool.tile([128, NQ * 4 * NW], f16, tag="sc")
                nfp_v = nfp_t[:, :].rearrange("p (c m) -> p c m", m=NW)
                for q in range(nq):
                    zt = psum.tile([128, MCHUNK], f32, tag="z",
                                   name=f"zt{t}_{q}")
                    for j in range(4):
                        cp = 2 * q + j // 2
                        ci = j % 2
                        lhsT = nfa_t[64 * ci:64 * ci + I1,
                                     cp * 128:(cp + 1) * 128]
                        rhs = ltv[64 * ci:64 * ci + I1, cp, :]
                        nc.tensor.matmul(zt[:, 512 * j:512 * j + NW],
                                         lhsT, rhs, start=True, stop=True)
                    zv = zt[:, :].rearrange(
                        "p (c n) -> p c n", n=512)[:, :, 0:NW]
                    rv = nfp_v[:, 4 * q:4 * q + 4, :]
                    ov = sct[:, q * 4 * NW:(q + 1) * 4 * NW].rearrange(
                        "p (c n) -> p c n", n=NW)
                    nc.vector._custom_dve(mac_scan, out=ov, in0=zv, in1=rv)

                # final: per 4c block: out_c0 = E0, out_cj = Ej - E(j-1)
                scv = sct[:, :].rearrange("p (q c m) -> p q c m",
                                          c=4, m=NW)
                ends = scv[:, :, :, NW - 1]
                ostf = finpool.tile([128, C], f32, tag="ostf")
                ostv = ostf[:, :].rearrange("p (q c) -> p q c", c=4)
                nc.vector.tensor_copy(ostv[:, :, 0:1], ends[:, :, 0:1])
                nc.vector.tensor_tensor(ostv[:, :, 1:4], ends[:, :, 1:4],
                                        ends[:, :, 0:3], sub)
                # undo the host-side 1/16 nfprod scaling (fp16 headroom)
                nc.vector.tensor_scalar_mul(ostf[:, :], ostf[:, :], 16.0)
                nc.gpsimd.dma_start(out=outp[t * 128:(t + 1) * 128, :],
                                    in_=ostf[:, :])
                if t + 1 < NT:
                    nfa_t = nfa_next
                    nfp_t = nfp_next
    nc.finalize()
    return nc


def _prep_inputs(node_feats, w3, w2, w1, U3, U2, U1):
    """Host-side fold, re-layout, fp16 casts, per-core sharding."""
    f16 = np.float16
    f32 = np.float32
    node_feats = np.asarray(node_feats, dtype=f32)

    # c-reorder for the build PSUM rows: [even c's | odd c's]
    c_perm = np.concatenate([np.arange(0, C, 2), np.arange(1, C, 2)])
    w3p = np.zeros((KP, C), dtype=f16)
    w3p[:K3] = np.asarray(w3, dtype=f32).astype(f16)[:, c_perm]
    # pre-swizzle to [p, kt*C] so the device load is contiguous
    w3p = np.ascontiguousarray(
        w3p.reshape(KT, 128, C).transpose(1, 0, 2).reshape(128, KT * C))
    w21 = np.zeros((32, C), dtype=f16)
    w21[:K2] = np.asarray(w2, dtype=f32).astype(f16)[:, c_perm]

    # nfa[p, t, cp, b]: p = 64*(c%2) + i'; i'=48 row is ones
    nf16 = node_feats.astype(f16)
    nfT = nf16.transpose(1, 2, 0)  # [c, i, b]
    nfa = np.zeros((128, NT, PAIRS, 128), dtype=f16)
    for par in (0, 1):
        s = nfT[par::2].transpose(1, 0, 2).reshape(I, PAIRS, NT, 128)
        nfa[64 * par:64 * par + I] = s.transpose(0, 2, 1, 3)
        nfa[64 * par + I] = 1.0
    nfa = np.ascontiguousarray(nfa.reshape(128, NT * PAIRS * 128))

    # fold U3: SYM[k, i, X, y] = U3w3-src folded over (x,y); diag halved
    U3_32 = np.asarray(U3, dtype=f32)
    u3_kixy = np.ascontiguousarray(U3_32.transpose(3, 2, 0, 1))
    SYM = (u3_kixy + u3_kixy.transpose(0, 1, 3, 2)).astype(f16)
    del u3_kixy
    U2f = np.asarray(U2, dtype=f32).transpose(2, 0, 1)
    U2S = (U2f + U2f.transpose(0, 2, 1)).astype(f16)

    # host U1 term (tiny): out1[b, c] = sum_x U1w1[c,x] nf[b,c,x]
    U1w1 = np.einsum("xk,kc->cx", np.asarray(U1, f32), np.asarray(w1, f32))
    host_out = np.einsum("cx,bcx->bc", U1w1,
                         node_feats.astype(np.float64))

    in_maps = []
    for r in range(NCORES):
        pairs = _core_pairs(r)
        u3a = np.zeros((KP, I, NW), dtype=f16)
        u2a = np.zeros((32, NW), dtype=f16)
        yidx = np.zeros(NW, dtype=np.int64)
        xidx = np.zeros(NW, dtype=np.int64)
        for g, (xa, xb) in enumerate(pairs):
            u3a[:K3, :, g * W:g * W + xa + 1] = SYM[:, :, xa, 0:xa + 1]
            u3a[:K3, :, g * W + xa] = SYM[:, :, xa, xa] / 2
            u2a[:K2, g * W:g * W + xa + 1] = U2S[:, xa, 0:xa + 1]
            u2a[:K2, g * W + xa] = U2S[:, xa, xa] / 2
            yidx[g * W:g * W + xa + 1] = np.arange(xa + 1)
            xidx[g * W:g * W + xa + 1] = xa
            u3a[:K3, :, g * W + xa + 1:g * W + W] = SYM[:, :, xb, 0:xb + 1]
            u3a[:K3, :, g * W + 48] = SYM[:, :, xb, xb] / 2
            u2a[:K2, g * W + xa + 1:g * W + W] = U2S[:, xb, 0:xb + 1]
            u2a[:K2, g * W + 48] = U2S[:, xb, xb] / 2
            yidx[g * W + xa + 1:g * W + W] = np.arange(xb + 1)
            xidx[g * W + xa + 1:g * W + W] = xb
        u3t = np.ascontiguousarray(u3a.reshape(KP, MP))

        # nfprod[b, c, w] = nf_y(w) * nf_X(w) / 16 (fp16 scan headroom;
        # the device rescales the final output by 16)
        nfprod = np.ascontiguousarray(
            (node_feats[:, :, yidx] * node_feats[:, :, xidx] * (1 / 16))
            .astype(f16).reshape(B, C * NW))

        in_maps.append({
            "u3t": u3t,
            "w3p": w3p,
            "nfa": nfa,
            "nfprod": nfprod,
            "u2aug": u2a,
            "w21": w21,
        })
    return in_maps, host_out


def kernel(node_feats, w3, w2, w1, U3, U2, U1):
    global LAST_EXEC_NS
    import os
    from concourse.bass_utils import run_bass_kernel_spmd

    if "nc" not in _CACHE:
        _CACHE["nc"] = _build_nc()
    nc = _CACHE["nc"]

    in_maps, host_out = _prep_inputs(node_feats, w3, w2, w1, U3, U2, U1)
    trace = bool(os.environ.get("BASS_TRACE"))
    res = run_bass_kernel_spmd(nc, in_maps, list(range(NCORES)),
                               trace=trace)
    LAST_EXEC_NS = res.exec_time_ns
    _CACHE["last_results"] = res

    out = host_out.copy()
    for r in range(NCORES):
        out += res.results[r]["out"].astype(np.float64)
    return out.astype(np.float32)


# revision 9
# speedup vs baseline: 1.1890x; 1.1890x over previous
"""Trainium2 Bass kernel for the MACE-style symmetric contraction:

    out  = einsum("xyik,kc,bci->bcxy", U3, w3, nf)
    c2   = einsum("xyk,kc->cxy", U2, w2)[None] + out
    out  = einsum("bcxi,bci->bcx", c2, nf)
    c1   = einsum("xk,kc->cx", U1, w1)[None] + out
    out  = einsum("bci,bci->bc", c1, nf)

Algebraically:

    out[b,c] =   sum_{x,y,i} W3U[x,y,i,c] nf[b,c,x] nf[b,c,y] nf[b,c,i]
               + sum_{x,y}   U2w2[c,x,y]  nf[b,c,x] nf[b,c,y]
               + sum_{x}     U1w1[c,x]    nf[b,c,x]

with W3U = einsum("xyik,kc->xyic", U3, w3).  Since nf_x*nf_y is symmetric
in (x,y), only the (x,y)-symmetric part of W3U/U2w2 contributes: fold the
rectangle onto unordered pairs {X, y<=X} via SYM[X,y] = W3U[X,y]+W3U[y,X]
(diagonal halved).  This halves the U3 HBM stream, the build matmul
columns, and the phase-B work vs the unfolded form.  The U1 term is added
on the host (tiny).

Sharding: each core owns 6 X-values {r, 15-r, 16+r, 31-r, 32+r, 47-r},
paired into 3 fold groups (Xa, Xb=47-Xa).  A group's 49 columns are
[Xa-run: y=0..Xa][Xb-run: y=0..Xb] - rectangular across cores, so one
SPMD program serves all cores; per-core structure lives in the data.

Queue layout (the point of the exercise: no queue mixes a potentially
blocking op into a streaming FIFO):
  sync   : u3 even k-tiles          scalar(ACT): u3 odd k-tiles
  gpsimd : scratch writes, lt piece loads (FIFO-ordered behind their
           producing scratch write), nfa/nfprod streams, out writes
  DVE    : PSUM drains (build), MAC-scans + final (phase B)

Device pipeline (fp16 data, fp32 PSUM/scan state):
  build:   A2[c, i', (g,w)] = w3.T @ u3s on PE, k-accumulated in PSUM,
           drained fp16 to a DRAM scratch (i'=48 row carries folded U2w2,
           contracted against a ones channel in nfa); the lt operand for
           phase B is re-loaded [i', c, w] piecewise as i-rows complete.
  phase B: per (8-c block, b-tile): Z[b,(g,w)] = nfa.T @ A2_c on PE into
           a [128,2048] PSUM tile (8 c's at 256-col offsets); one fused
           DVE MAC-scan against the host-streamed weight tensor
           nfprod[b,c,w] = nf_y(w)*nf_X(w)/16 accumulates Z*nfprod, so
           the scan value at the end of each c's 147 columns IS out[b,c]
           (recovered by a shifted subtract, then rescaled by 16).
"""

import numpy as np

B = 512          # atoms
C = 96           # feats
I = 48           # irreps
K3, K2, K1 = 1270, 24, 3
NCORES = 8
I1 = I + 1        # 49 contraction rows (i + U2 aug row)
W = 49            # folded group width
G = 3             # fold groups per core
NW = G * W        # 147 columns per core
MP = I * NW       # 7056 build m-columns (m = i*NW + g*W + w)
SCR = I1 * NW     # 7203 scratch cols per c (aug row at 7056..7202)
KP = 1280         # K3 padded to 10 partition tiles
KT = KP // 128    # 10
MCHUNK = 2048     # build chunk (one PSUM tile, 4 banks)
PAIRS = C // 2    # 48
NT = B // 128     # 4 b-tiles
NQ = C // 8       # 12 phase-B units per b-tile (8 c's each)

_CACHE = {}

# exec time of the last device run (ns), when BASS_TRACE=1
LAST_EXEC_NS = None


def _core_pairs(r):
    """Fold pairs (Xa, Xb) with Xa+Xb=47; Xa-run first (y=0..Xa)."""
    return [(r, 47 - r), (15 - r, 32 + r), (16 + r, 31 - r)]


def _register_mac_scan():
    """Custom DVE op: out[t] = prefix-sum of in0[t]*in1[t] (fp32 state)."""
    import concourse.dve_ops as dve_ops_mod
    if any(op.name == "TT_MAC_SCAN_ANT" for op in dve_ops_mod.OPS):
        return next(op for op in dve_ops_mod.OPS
                    if op.name == "TT_MAC_SCAN_ANT")
    from concourse.dve_spec import Spec, scan, Src0, Src1
    from concourse.dve_uop import AluOp
    from concourse.dve_ops import DveOp

    def _ref_mac_scan(in0, in1, s0, s1, imm2):
        p = in0.astype(np.float32) * in1.astype(np.float32)
        return np.cumsum(p.reshape(p.shape[0], -1), axis=1).reshape(
            p.shape).astype(np.float32)

    spec = Spec(body=scan(AluOp.ADD, Src0 * Src1), reference=_ref_mac_scan)
    op = DveOp("TT_MAC_SCAN_ANT", spec, subdim=False,
               uops_sha={"v3": "b3fc3e78a862b7eb",
                         "v4": "bc6a002865d48b97"})
    dve_ops_mod.OPS.append(op)
    dve_ops_mod.CUSTOM_DVE_SPECS[op.name] = spec
    dve_ops_mod._SUB_OPCODE_FOR_NAME[op.name] = (
        max(dve_ops_mod._SUB_OPCODE_FOR_NAME.values()) + 1)
    return op


def _build_nc(debug=None):
    import concourse.mybir as mybir
    from concourse.tile import TileContext

    mac_scan = _register_mac_scan()

    f16 = mybir.dt.float16
    f32 = mybir.dt.float32
    sub = mybir.AluOpType.subtract

    import concourse.bacc as bacc
    nc = bacc.Bacc(None, target_bir_lowering=False)
    u3t = nc.dram_tensor("u3t", [KP, MP], f16, kind="ExternalInput")
    # w3p pre-swizzled host-side to [p, kt*C] so the load is contiguous;
    # its c-axis is reordered [even c's | odd c's] so the build PSUM rows
    # land even-c in rows 0..47, odd-c in 48..95.
    w3p = nc.dram_tensor("w3p", [128, KT * C], f16, kind="ExternalInput")
    # nfa[p, t, cp, b128]: p = 64*(c%2) + i'; i'=48 row is ones
    nfa = nc.dram_tensor("nfa", [128, NT * PAIRS * 128], f16,
                         kind="ExternalInput")
    # nfprod[b, c*NW]: per-column weight nf_y(w)*nf_X(w)/16
    nfprod = nc.dram_tensor("nfprod", [B, C * NW], f16,
                            kind="ExternalInput")
    u2aug = nc.dram_tensor("u2aug", [32, NW], f16, kind="ExternalInput")
    w21 = nc.dram_tensor("w21", [32, C], f16, kind="ExternalInput")
    outp = nc.dram_tensor("out", [B, C], f32, kind="ExternalOutput")

    with TileContext(nc) as tc:
        with (
            nc.allow_low_precision(reason="fp16 intermediates; rel-err "
                                   "budget 2e-2 vs ~1e-3 incurred"),
            tc.tile_pool(name="dram", bufs=1, space="DRAM") as dpool,
            tc.tile_pool(name="const", bufs=1) as cpool,
            tc.tile_pool(name="u3", bufs=6) as u3pool,
            tc.tile_pool(name="psum", bufs=2, space="PSUM") as psum,
            tc.tile_pool(name="sc", bufs=2) as scpool,
            tc.tile_pool(name="nfr", bufs=2) as nfrpool,
            tc.tile_pool(name="nfat", bufs=2) as nfapool,
            tc.tile_pool(name="stg", bufs=3) as stgpool,
            tc.tile_pool(name="fin", bufs=2) as finpool,
        ):
            # scratch row c = [i-major build cols 0..7055 | aug 7056..7202]
            # rows: even c's at 0..47, odd at 48..95 (w3p reorder)
            w3u_scr = dpool.tile([C, SCR], f16)

            w3sb = cpool.tile([128, KT * C], f16)
            nc.sync.dma_start(out=w3sb[:, :], in_=w3p[:, :])
            w3v = w3sb[:, :].rearrange("p (k c) -> p k c", c=C)
            w21sb = cpool.tile([32, C], f16)
            nc.sync.dma_start(out=w21sb[:, :], in_=w21[:, :])
            u2sb = cpool.tile([32, NW], f16)
            nc.sync.dma_start(out=u2sb[:, :], in_=u2aug[:, :])

            nfav = nfa[:, :].rearrange("p (t m) -> p t m", t=NT)

            def load_nfa(t):
                nt = nfapool.tile([128, PAIRS * 128], f16, tag="nfa")
                for par in (0, 1):
                    r0 = 64 * par
                    nc.gpsimd.dma_start(out=nt[r0:r0 + I1, :],
                                        in_=nfav[r0:r0 + I1, t, :])
                return nt

            def load_nfprod(t):
                nt = nfrpool.tile([128, C * NW], f16, tag="nfr")
                nc.gpsimd.dma_start(out=nt[:, :],
                                    in_=nfprod[t * 128:(t + 1) * 128, :])
                return nt

            nfa_t = load_nfa(0)
            nfp_t = load_nfprod(0)

            # lt: one resident tile; loaded piecewise as the scratch
            # completes (even-c c0..c47 to rows 0..48, odd-c to 64..112)
            lt = cpool.tile([128, PAIRS * NW], f16, name="ltbig")
            ltv = lt[:, :].rearrange("p (c m) -> p c m", m=NW)
            scr3 = w3u_scr[:, :].rearrange("c (i m) -> c i m", m=NW)

            def load_lt_rows(i_lo, i_hi):
                # gpsimd queue: FIFO-ordered behind the scratch write that
                # produced these rows, so no extra sync is needed.
                for par in (0, 1):
                    nc.gpsimd.dma_start(
                        out=ltv[64 * par + i_lo:64 * par + i_hi, :, :],
                        in_=scr3[48 * par:48 * par + 48,
                                 i_lo:i_hi, :].rearrange(
                                     "c i m -> i c m"))

            # ---- aug build: [96, 147] = w21.T @ u2aug -> aug row ----
            aps = psum.tile([128, MCHUNK], f32, tag="z")
            nc.tensor.matmul(aps[:C, :NW], w21sb[:K2, :], u2sb[:K2, :],
                             start=True, stop=True)
            astg = stgpool.tile([C, MCHUNK], f16, tag="stg")
            nc.vector.tensor_copy(astg[:, :NW], aps[:C, :NW])
            nc.gpsimd.dma_start(out=w3u_scr[:, MP:SCR], in_=astg[:, :NW])
            load_lt_rows(I, I1)  # aug row

            # ---- build: A2[c, m] = w3.T @ u3s, k-accumulated; u3 tiles
            # alternate sync/scalar queues to hide per-DMA gaps (both
            # queues carry nothing else during the build) ----
            NMC = (MP + MCHUNK - 1) // MCHUNK  # 4 (2048,2048,2048,912)
            i_done = 0
            for mcp in range(NMC):
                wc = min(MCHUNK, MP - mcp * MCHUNK)
                ps = psum.tile([128, MCHUNK], f32, tag="z",
                               name=f"bp{mcp}")
                for kt in range(KT):
                    tl = u3pool.tile([128, MCHUNK], f16, tag="u3")
                    base = mcp * MCHUNK
                    eng = nc.sync if kt % 2 == 0 else nc.scalar
                    eng.dma_start(
                        out=tl[:, :wc],
                        in_=u3t[kt * 128:(kt + 1) * 128, base:base + wc])
                    for off in range(0, wc, 512):
                        h = min(512, wc - off)
                        nc.tensor.matmul(
                            ps[:C, off:off + h], w3v[:, kt, :],
                            tl[:, off:off + h],
                            start=(kt == 0), stop=(kt == KT - 1))
                stg = stgpool.tile([C, MCHUNK], f16, tag="stg")
                nc.vector.tensor_copy(stg[:, :wc], ps[:C, :wc])
                nc.gpsimd.dma_start(
                    out=w3u_scr[:, mcp * MCHUNK:mcp * MCHUNK + wc],
                    in_=stg[:, :wc])
                i_new = ((mcp + 1) * MCHUNK) // NW if mcp + 1 < NMC else I
                if i_new > i_done:
                    load_lt_rows(i_done, i_new)
                    i_done = i_new

            # ---- phase B, t-major; unit = 8 consecutive c's in one
            # [128, 2048] PSUM tile at 256-col offsets ----
            nq = NQ if debug is None else debug
            for t in range(NT):
                if t + 1 < NT:
                    nfa_next = load_nfa(t + 1)
                    nfp_next = load_nfprod(t + 1)
                sct = scpool.tile([128, NQ * 8 * NW], f16, tag="sc")
                nfp_v = nfp_t[:, :].rearrange("p (c m) -> p c m", m=NW)
                for q2 in range(2 * nq):
                    zt = psum.tile([128, MCHUNK], f32, tag="z",
                                   name=f"zt{t}_{q2}")
                    for j in range(4):
                        cp = 2 * q2 + j // 2
                        ci = j % 2
                        lhsT = nfa_t[64 * ci:64 * ci + I1,
                                     cp * 128:(cp + 1) * 128]
                        rhs = ltv[64 * ci:64 * ci + I1, cp, :]
                        nc.tensor.matmul(zt[:, 512 * j:512 * j + NW],
                                         lhsT, rhs, start=True, stop=True)
                    zv = zt[:, :].rearrange(
                        "p (c n) -> p c n", n=512)[:, :, 0:NW]
                    rv = nfp_v[:, 4 * q2:4 * q2 + 4, :]
                    ov = sct[:, q2 * 4 * NW:(q2 + 1) * 4 * NW].rearrange(
                        "p (c n) -> p c n", n=NW)
                    nc.vector._custom_dve(mac_scan, out=ov, in0=zv, in1=rv)

                # final: per 4c block: out_c0 = E0, out_cj = Ej - E(j-1)
                scv = sct[:, :].rearrange("p (q c m) -> p q c m",
                                          c=4, m=NW)
                ends = scv[:, :, :, NW - 1]
                ostf = finpool.tile([128, C], f32, tag="ostf")
                ostv = ostf[:, :].rearrange("p (q c) -> p q c", c=4)
                nc.vector.tensor_copy(ostv[:, :, 0:1], ends[:, :, 0:1])
                nc.vector.tensor_tensor(ostv[:, :, 1:4], ends[:, :, 1:4],
                                        ends[:, :, 0:3], sub)
                # undo the host-side 1/16 nfprod scaling (fp16 headroom)
                nc.vector.tensor_scalar_mul(ostf[:, :], ostf[:, :], 16.0)
                nc.gpsimd.dma_start(out=outp[t * 128:(t + 1) * 128, :],
                                    in_=ostf[:, :])
                if t + 1 < NT:
                    nfa_t = nfa_next
                    nfp_t = nfp_next
    nc.finalize()
    return nc


def _prep_inputs(node_feats, w3, w2, w1, U3, U2, U1):
    """Host-side fold, re-layout, fp16 casts, per-core sharding."""
    f16 = np.float16
    f32 = np.float32
    node_feats = np.asarray(node_feats, dtype=f32)

    # c-reorder for the build PSUM rows: [even c's | odd c's]
    c_perm = np.concatenate([np.arange(0, C, 2), np.arange(1, C, 2)])
    w3p = np.zeros((KP, C), dtype=f16)
    w3p[:K3] = np.asarray(w3, dtype=f32).astype(f16)[:, c_perm]
    # pre-swizzle to [p, kt*C] so the device load is contiguous
    w3p = np.ascontiguousarray(
        w3p.reshape(KT, 128, C).transpose(1, 0, 2).reshape(128, KT * C))
    w21 = np.zeros((32, C), dtype=f16)
    w21[:K2] = np.asarray(w2, dtype=f32).astype(f16)[:, c_perm]

    # nfa[p, t, cp, b]: p = 64*(c%2) + i'; i'=48 row is ones
    nf16 = node_feats.astype(f16)
    nfT = nf16.transpose(1, 2, 0)  # [c, i, b]
    nfa = np.zeros((128, NT, PAIRS, 128), dtype=f16)
    for par in (0, 1):
        s = nfT[par::2].transpose(1, 0, 2).reshape(I, PAIRS, NT, 128)
        nfa[64 * par:64 * par + I] = s.transpose(0, 2, 1, 3)
        nfa[64 * par + I] = 1.0
    nfa = np.ascontiguousarray(nfa.reshape(128, NT * PAIRS * 128))

    # fold U3: SYM[k, i, X, y] = U3w3-src folded over (x,y); diag halved
    U3_32 = np.asarray(U3, dtype=f32)
    u3_kixy = np.ascontiguousarray(U3_32.transpose(3, 2, 0, 1))
    SYM = (u3_kixy + u3_kixy.transpose(0, 1, 3, 2)).astype(f16)
    del u3_kixy
    U2f = np.asarray(U2, dtype=f32).transpose(2, 0, 1)
    U2S = (U2f + U2f.transpose(0, 2, 1)).astype(f16)

    # host U1 term (tiny): out1[b, c] = sum_x U1w1[c,x] nf[b,c,x]
    U1w1 = np.einsum("xk,kc->cx", np.asarray(U1, f32), np.asarray(w1, f32))
    host_out = np.einsum("cx,bcx->bc", U1w1,
                         node_feats.astype(np.float64))

    in_maps = []
    for r in range(NCORES):
        pairs = _core_pairs(r)
        u3a = np.zeros((KP, I, NW), dtype=f16)
        u2a = np.zeros((32, NW), dtype=f16)
        yidx = np.zeros(NW, dtype=np.int64)
        xidx = np.zeros(NW, dtype=np.int64)
        for g, (xa, xb) in enumerate(pairs):
            u3a[:K3, :, g * W:g * W + xa + 1] = SYM[:, :, xa, 0:xa + 1]
            u3a[:K3, :, g * W + xa] = SYM[:, :, xa, xa] / 2
            u2a[:K2, g * W:g * W + xa + 1] = U2S[:, xa, 0:xa + 1]
            u2a[:K2, g * W + xa] = U2S[:, xa, xa] / 2
            yidx[g * W:g * W + xa + 1] = np.arange(xa + 1)
            xidx[g * W:g * W + xa + 1] = xa
            u3a[:K3, :, g * W + xa + 1:g * W + W] = SYM[:, :, xb, 0:xb + 1]
            u3a[:K3, :, g * W + 48] = SYM[:, :, xb, xb] / 2
            u2a[:K2, g * W + xa + 1:g * W + W] = U2S[:, xb, 0:xb + 1]
            u2a[:K2, g * W + 48] = U2S[:, xb, xb] / 2
            yidx[g * W + xa + 1:g * W + W] = np.arange(xb + 1)
            xidx[g * W + xa + 1:g * W + W] = xb
        u3t = np.ascontiguousarray(u3a.reshape(KP, MP))

        # nfprod[b, c, w] = nf_y(w) * nf_X(w) / 16 (fp16 scan headroom;
        # the device rescales the final output by 16)
        nfprod = np.ascontiguousarray(
            (node_feats[:, :, yidx] * node_feats[:, :, xidx] * (1 / 16))
            .astype(f16).reshape(B, C * NW))

        in_maps.append({
            "u3t": u3t,
            "w3p": w3p,
            "nfa": nfa,
            "nfprod": nfprod,
            "u2aug": u2a,
            "w21": w21,
        })
    return in_maps, host_out


def kernel(node_feats, w3, w2, w1, U3, U2, U1):
    global LAST_EXEC_NS
    import os
    from concourse.bass_utils import run_bass_kernel_spmd

    if "nc" not in _CACHE:
        _CACHE["nc"] = _build_nc()
    nc = _CACHE["nc"]

    in_maps, host_out = _prep_inputs(node_feats, w3, w2, w1, U3, U2, U1)
    trace = bool(os.environ.get("BASS_TRACE"))
    res = run_bass_kernel_spmd(nc, in_maps, list(range(NCORES)),
                               trace=trace)
    LAST_EXEC_NS = res.exec_time_ns
    _CACHE["last_results"] = res

    out = host_out.copy()
    for r in range(NCORES):
        out += res.results[r]["out"].astype(np.float64)
    return out.astype(np.float32)


# revision 10
# speedup vs baseline: 1.2690x; 1.0673x over previous
"""Trainium2 Bass kernel for the MACE-style symmetric contraction:

    out  = einsum("xyik,kc,bci->bcxy", U3, w3, nf)
    c2   = einsum("xyk,kc->cxy", U2, w2)[None] + out
    out  = einsum("bcxi,bci->bcx", c2, nf)
    c1   = einsum("xk,kc->cx", U1, w1)[None] + out
    out  = einsum("bci,bci->bc", c1, nf)

Algebraically:

    out[b,c] =   sum_{x,y,i} W3U[x,y,i,c] nf[b,c,x] nf[b,c,y] nf[b,c,i]
               + sum_{x,y}   U2w2[c,x,y]  nf[b,c,x] nf[b,c,y]
               + sum_{x}     U1w1[c,x]    nf[b,c,x]

with W3U = einsum("xyik,kc->xyic", U3, w3).  The weight nf_x nf_y nf_i is
FULLY symmetric in (x,y,i), so only the symmetrized U3 coefficient on
multiset-canonical triples y <= X <= i matters: SYM6 = sum of U3w3-source
over all 6 axis permutations, divided by 2 where exactly two indices
coincide and 6 on the triple diagonal.  That cuts the U3 HBM stream and
the build matmul columns ~4x vs the naive rectangle (after padding).
U2w2 is folded the same way over its (x,y) pair; U1w1 is added on the
host (tiny).

Sharding: each core owns 6 X-values {r, 15-r, 16+r, 31-r, 32+r, 47-r} -
exactly one per 8-wide X-band, so each core's 6 canonical runs fit the
SAME 6 padded rectangles [H=48-8j rows(i), L=8(j+1) cols(y)] across all
cores: one SPMD program, per-core structure entirely in the data (pad
entries are zeros).  Host sums the 8 partial outputs.

Queue layout (no queue mixes a potentially blocking op into a streaming
FIFO):
  sync / scalar(ACT): u3 k-quad tiles, alternating
  gpsimd : scratch writes, lt piece loads (FIFO-ordered behind their
           producing scratch write), nfa/nfprod streams, out writes
  DVE    : PSUM drains (build), MAC-scans + final (phase B)

Device pipeline (fp16 data, fp32 PSUM/scan state):
  build:   A3[c, slot rects] = w3.T @ u3s on PE, k-accumulated in PSUM,
           drained fp16 to a DRAM scratch; re-loaded per slot as
           lt[i-row, c, w-col] (i'=48 row carries folded U2w2,
           contracted against a ones channel in nfa).
  phase B: per (4-c block, b-tile): Z[b,w] = nfa.T @ lt_c on PE into a
           [128,2048] PSUM tile (4 c's at 512-col offsets); one fused
           DVE MAC-scan against the host-streamed weight tensor
           nfprod[b,c,w] = nf_y(w)*nf_X(w)/16 accumulates Z*nfprod, so
           the scan value at the end of each c's 168 columns IS out[b,c]
           (recovered by a shifted subtract, then rescaled by 16).
"""

import numpy as np

B = 512          # atoms
C = 96           # feats
I = 48           # irreps
K3, K2, K1 = 1270, 24, 3
NCORES = 8
I1 = I + 1        # 49 contraction rows (i + U2 aug row)
NW = 168          # padded w-columns per c (6 slots, L = 8,16,..,48)
HS = [48, 40, 32, 24, 16, 8]      # slot i-row heights (rows 8j..47)
LSZ = [8, 16, 24, 32, 40, 48]     # slot y-widths
CUML = [0, 8, 24, 48, 80, 120]    # slot w-column bases
SIZES = [h * l for h, l in zip(HS, LSZ)]   # 384,640,768,768,640,384
MBASE = [0, 384, 1024, 1792, 2560, 3200]
MP = 3584         # packed build m-columns
SCR = MP + NW     # 3752 scratch cols per c (aug row area at the tail)
KP = 1280         # K3 padded to 10 partition tiles
KT = KP // 128    # 10
PAIRS = C // 2    # 48
NT = B // 128     # 4 b-tiles

_CACHE = {}

# exec time of the last device run (ns), when BASS_TRACE=1
LAST_EXEC_NS = None


def _core_xset(r):
    return sorted([r, 15 - r, 16 + r, 31 - r, 32 + r, 47 - r])


def _register_mac_scan():
    """Custom DVE op: out[t] = prefix-sum of in0[t]*in1[t] (fp32 state)."""
    import concourse.dve_ops as dve_ops_mod
    if any(op.name == "TT_MAC_SCAN_ANT" for op in dve_ops_mod.OPS):
        return next(op for op in dve_ops_mod.OPS
                    if op.name == "TT_MAC_SCAN_ANT")
    from concourse.dve_spec import Spec, scan, Src0, Src1
    from concourse.dve_uop import AluOp
    from concourse.dve_ops import DveOp

    def _ref_mac_scan(in0, in1, s0, s1, imm2):
        p = in0.astype(np.float32) * in1.astype(np.float32)
        return np.cumsum(p.reshape(p.shape[0], -1), axis=1).reshape(
            p.shape).astype(np.float32)

    spec = Spec(body=scan(AluOp.ADD, Src0 * Src1), reference=_ref_mac_scan)
    op = DveOp("TT_MAC_SCAN_ANT", spec, subdim=False,
               uops_sha={"v3": "b3fc3e78a862b7eb",
                         "v4": "bc6a002865d48b97"})
    dve_ops_mod.OPS.append(op)
    dve_ops_mod.CUSTOM_DVE_SPECS[op.name] = spec
    dve_ops_mod._SUB_OPCODE_FOR_NAME[op.name] = (
        max(dve_ops_mod._SUB_OPCODE_FOR_NAME.values()) + 1)
    return op


def _build_nc(debug=None):
    import concourse.mybir as mybir
    from concourse.tile import TileContext

    mac_scan = _register_mac_scan()

    f16 = mybir.dt.float16
    f32 = mybir.dt.float32
    sub = mybir.AluOpType.subtract

    import concourse.bacc as bacc
    nc = bacc.Bacc(None, target_bir_lowering=False)
    u3t = nc.dram_tensor("u3t", [KP, MP], f16, kind="ExternalInput")
    # w3p pre-swizzled host-side to [p, kt*C]; its c-axis is reordered
    # [even c's | odd c's] so build PSUM rows land even-c in rows 0..47.
    w3p = nc.dram_tensor("w3p", [128, KT * C], f16, kind="ExternalInput")
    # nfa[p, t, cp, b128]: p = 64*(c%2) + i'; i'=48 row is ones
    nfa = nc.dram_tensor("nfa", [128, NT * PAIRS * 128], f16,
                         kind="ExternalInput")
    # nfprod[b, c*NW]: per-column weight nf_y(w)*nf_X(w)/16 (0 on pads)
    nfprod = nc.dram_tensor("nfprod", [B, C * NW], f16,
                            kind="ExternalInput")
    u2aug = nc.dram_tensor("u2aug", [32, NW], f16, kind="ExternalInput")
    w21 = nc.dram_tensor("w21", [32, C], f16, kind="ExternalInput")
    outp = nc.dram_tensor("out", [B, C], f32, kind="ExternalOutput")

    with TileContext(nc) as tc:
        with (
            nc.allow_low_precision(reason="fp16 intermediates; rel-err "
                                   "budget 2e-2 vs ~1e-3 incurred"),
            tc.tile_pool(name="dram", bufs=1, space="DRAM") as dpool,
            tc.tile_pool(name="const", bufs=1) as cpool,
            tc.tile_pool(name="u3", bufs=4) as u3pool,
            tc.tile_pool(name="psum", bufs=2, space="PSUM") as psum,
            tc.tile_pool(name="sc", bufs=2) as scpool,
            tc.tile_pool(name="nfr", bufs=2) as nfrpool,
            tc.tile_pool(name="nfat", bufs=2) as nfapool,
            tc.tile_pool(name="stg", bufs=3) as stgpool,
            tc.tile_pool(name="fin", bufs=2) as finpool,
        ):
            # scratch row c = [6 slot rects | aug row area]
            # rows: even c's at 0..47, odd at 48..95 (w3p reorder)
            w3u_scr = dpool.tile([C, SCR], f16)

            w3sb = cpool.tile([128, KT * C], f16)
            nc.sync.dma_start(out=w3sb[:, :], in_=w3p[:, :])
            w3v = w3sb[:, :].rearrange("p (k c) -> p k c", c=C)
            w21sb = cpool.tile([32, C], f16)
            nc.sync.dma_start(out=w21sb[:, :], in_=w21[:, :])
            u2sb = cpool.tile([32, NW], f16)
            nc.sync.dma_start(out=u2sb[:, :], in_=u2aug[:, :])

            nfav = nfa[:, :].rearrange("p (t m) -> p t m", t=NT)

            def load_nfa(t):
                nt = nfapool.tile([128, PAIRS * 128], f16, tag="nfa")
                for par in (0, 1):
                    r0 = 64 * par
                    nc.gpsimd.dma_start(out=nt[r0:r0 + I1, :],
                                        in_=nfav[r0:r0 + I1, t, :])
                return nt

            def load_nfprod(t):
                nt = nfrpool.tile([128, C * NW], f16, tag="nfr")
                nc.gpsimd.dma_start(out=nt[:, :],
                                    in_=nfprod[t * 128:(t + 1) * 128, :])
                return nt

            nfa_t = load_nfa(0)
            nfp_t = load_nfprod(0)

            # lt[p, c, w]: rows = i (0..47) + aug row 48 (offset 64 for
            # odd c).  Slots only write rows 8j..47 of their column band;
            # everything else must read as zero.
            lt = cpool.tile([128, PAIRS * NW], f16, name="ltbig")
            nc.vector.memset(lt[:, :], 0.0)
            ltv = lt[:, :].rearrange("p (c m) -> p c m", m=NW)

            # ---- aug build: [96, 168] = w21.T @ u2aug -> aug row ----
            aps = psum.tile([128, 2048], f32, tag="z")
            nc.tensor.matmul(aps[:C, :NW], w21sb[:K2, :], u2sb[:K2, :],
                             start=True, stop=True)
            astg = stgpool.tile([C, 768], f16, tag="stg")
            nc.vector.tensor_copy(astg[:, :NW], aps[:C, :NW])
            nc.gpsimd.dma_start(out=w3u_scr[:, MP:SCR], in_=astg[:, :NW])
            for par in (0, 1):  # aug -> lt row 48
                nc.gpsimd.dma_start(
                    out=ltv[64 * par + I:64 * par + I1, :, :],
                    in_=w3u_scr[48 * par:48 * par + 48,
                                MP:SCR].rearrange("c (h m) -> h c m", h=1))

            # ---- build: per slot j, A3 rect = w3.T @ u3s, k-accumulated;
            # u3 k-quad tiles alternate sync/scalar queues ----
            for j in range(6):
                size = SIZES[j]
                ps = psum.tile([128, 2048], f32, tag="z", name=f"bp{j}")
                kt = 0
                qi = 0
                while kt < KT:
                    g = min(4, KT - kt)
                    tl = u3pool.tile([128, 4 * 768], f16, tag="u3")
                    tlv = tl[:, :].rearrange("p (g m) -> p g m", m=768)
                    eng = nc.sync if qi % 2 == 0 else nc.scalar
                    eng.dma_start(
                        out=tlv[:, 0:g, 0:size],
                        in_=u3t[kt * 128:(kt + g) * 128,
                                MBASE[j]:MBASE[j] + size].rearrange(
                                    "(g p) m -> p g m", p=128))
                    for gg in range(g):
                        for off in range(0, size, 512):
                            h = min(512, size - off)
                            nc.tensor.matmul(
                                ps[:C, off:off + h],
                                w3v[:, kt + gg, :],
                                tlv[:, gg, off:off + h],
                                start=(kt + gg == 0),
                                stop=(kt + gg == KT - 1))
                    kt += g
                    qi += 1
                stg = stgpool.tile([C, 768], f16, tag="stg")
                nc.vector.tensor_copy(stg[:, :size], ps[:C, :size])
                nc.gpsimd.dma_start(
                    out=w3u_scr[:, MBASE[j]:MBASE[j] + size],
                    in_=stg[:, :size])
                # lt piece: rows 8j..47, column band CUML[j]..+LSZ[j]
                for par in (0, 1):
                    nc.gpsimd.dma_start(
                        out=ltv[64 * par + 8 * j:64 * par + I, :,
                                CUML[j]:CUML[j] + LSZ[j]],
                        in_=w3u_scr[48 * par:48 * par + 48,
                                    MBASE[j]:MBASE[j] + size].rearrange(
                                        "c (h l) -> h c l", l=LSZ[j]))

            # ---- phase B, t-major; unit = 4 consecutive c's in one
            # [128, 2048] PSUM tile at 512-col offsets ----
            nq = PAIRS // 2 if debug is None else debug
            for t in range(NT):
                if t + 1 < NT:
                    nfa_next = load_nfa(t + 1)
                    nfp_next = load_nfprod(t + 1)
                sct = scpool.tile([128, C * NW], f16, tag="sc")
                nfp_v = nfp_t[:, :].rearrange("p (c m) -> p c m", m=NW)
                for q2 in range(nq):
                    zt = psum.tile([128, 2048], f32, tag="z",
                                   name=f"zt{t}_{q2}")
                    for j in range(4):
                        cp = 2 * q2 + j // 2
                        ci = j % 2
                        lhsT = nfa_t[64 * ci:64 * ci + I1,
                                     cp * 128:(cp + 1) * 128]
                        rhs = ltv[64 * ci:64 * ci + I1, cp, :]
                        nc.tensor.matmul(zt[:, 512 * j:512 * j + NW],
                                         lhsT, rhs, start=True, stop=True)
                    zv = zt[:, :].rearrange(
                        "p (c n) -> p c n", n=512)[:, :, 0:NW]
                    rv = nfp_v[:, 4 * q2:4 * q2 + 4, :]
                    ov = sct[:, q2 * 4 * NW:(q2 + 1) * 4 * NW].rearrange(
                        "p (c n) -> p c n", n=NW)
                    nc.vector._custom_dve(mac_scan, out=ov, in0=zv, in1=rv)

                # final: per 4c block: out_c0 = E0, out_cj = Ej - E(j-1)
                scv = sct[:, :].rearrange("p (q c m) -> p q c m",
                                          c=4, m=NW)
                ends = scv[:, :, :, NW - 1]
                ostf = finpool.tile([128, C], f32, tag="ostf")
                ostv = ostf[:, :].rearrange("p (q c) -> p q c", c=4)
                nc.vector.tensor_copy(ostv[:, :, 0:1], ends[:, :, 0:1])
                nc.vector.tensor_tensor(ostv[:, :, 1:4], ends[:, :, 1:4],
                                        ends[:, :, 0:3], sub)
                # undo the host-side 1/16 nfprod scaling (fp16 headroom)
                nc.vector.tensor_scalar_mul(ostf[:, :], ostf[:, :], 16.0)
                nc.gpsimd.dma_start(out=outp[t * 128:(t + 1) * 128, :],
                                    in_=ostf[:, :])
                if t + 1 < NT:
                    nfa_t = nfa_next
                    nfp_t = nfp_next
    nc.finalize()
    return nc


def _prep_inputs(node_feats, w3, w2, w1, U3, U2, U1):
    """Host-side symmetrization, re-layout, fp16 casts, sharding."""
    f16 = np.float16
    f32 = np.float32
    node_feats = np.asarray(node_feats, dtype=f32)

    # c-reorder for the build PSUM rows: [even c's | odd c's]
    c_perm = np.concatenate([np.arange(0, C, 2), np.arange(1, C, 2)])
    w3p = np.zeros((KP, C), dtype=f16)
    w3p[:K3] = np.asarray(w3, dtype=f32).astype(f16)[:, c_perm]
    w3p = np.ascontiguousarray(
        w3p.reshape(KT, 128, C).transpose(1, 0, 2).reshape(128, KT * C))
    w21 = np.zeros((32, C), dtype=f16)
    w21[:K2] = np.asarray(w2, dtype=f32).astype(f16)[:, c_perm]

    # nfa[p, t, cp, b]: p = 64*(c%2) + i'; i'=48 row is ones
    nf16 = node_feats.astype(f16)
    nfT = nf16.transpose(1, 2, 0)  # [c, i, b]
    nfa = np.zeros((128, NT, PAIRS, 128), dtype=f16)
    for par in (0, 1):
        s = nfT[par::2].transpose(1, 0, 2).reshape(I, PAIRS, NT, 128)
        nfa[64 * par:64 * par + I] = s.transpose(0, 2, 1, 3)
        nfa[64 * par + I] = 1.0
    nfa = np.ascontiguousarray(nfa.reshape(128, NT * PAIRS * 128))

    # SYM6[k, i, x, y] = sum over all 6 permutations of (i, x, y)
    U3_32 = np.asarray(U3, dtype=f32)
    Tk = np.ascontiguousarray(U3_32.transpose(3, 2, 0, 1))  # [k, i, x, y]
    SYM6 = (Tk + Tk.transpose(0, 1, 3, 2) + Tk.transpose(0, 2, 1, 3)
            + Tk.transpose(0, 2, 3, 1) + Tk.transpose(0, 3, 1, 2)
            + Tk.transpose(0, 3, 2, 1))
    del Tk
    U2f = np.asarray(U2, dtype=f32).transpose(2, 0, 1)
    U2S = (U2f + U2f.transpose(0, 2, 1)).astype(f16)

    # host U1 term (tiny): out1[b, c] = sum_x U1w1[c,x] nf[b,c,x]
    U1w1 = np.einsum("xk,kc->cx", np.asarray(U1, f32), np.asarray(w1, f32))
    host_out = np.einsum("cx,bcx->bc", U1w1,
                         node_feats.astype(np.float64))

    in_maps = []
    for r in range(NCORES):
        Xs = _core_xset(r)
        u3p = np.zeros((KP, MP), dtype=f16)
        u2a = np.zeros((32, NW), dtype=f16)
        yidx = np.zeros(NW, dtype=np.int64)
        xidx = np.zeros(NW, dtype=np.int64)
        mask = np.zeros(NW, dtype=f32)
        for j, X in enumerate(Xs):
            H, L, base = HS[j], LSZ[j], CUML[j]
            # canonical entries y <= X <= i with multiplicity fix
            blk = SYM6[:K3, X:48, X, 0:X + 1].copy()   # [k, i-X, y]
            blk[:, :, X] /= 2                           # y == X
            blk[:, 0, :] /= 2                           # i == X
            blk[:, 0, X] = SYM6[:K3, X, X, X] / 6       # y == X == i
            slot = np.zeros((KP, H, L), dtype=f16)
            slot[:K3, X - 8 * j:, 0:X + 1] = blk        # row p = i - 8j
            u3p[:, MBASE[j]:MBASE[j] + H * L] = slot.reshape(KP, H * L)
            u2a[:K2, base:base + X + 1] = U2S[:, X, 0:X + 1]
            u2a[:K2, base + X] = U2S[:, X, X] / 2
            yidx[base:base + X + 1] = np.arange(X + 1)
            xidx[base:base + X + 1] = X
            mask[base:base + X + 1] = 1.0

        # nfprod[b, c, w] = nf_y(w) * nf_X(w) / 16, zero on pad columns
        nfprod = np.ascontiguousarray(
            (node_feats[:, :, yidx] * node_feats[:, :, xidx]
             * (mask / 16)).astype(f16).reshape(B, C * NW))

        in_maps.append({
            "u3t": u3p,
            "w3p": w3p,
            "nfa": nfa,
            "nfprod": nfprod,
            "u2aug": u2a,
            "w21": w21,
        })
    return in_maps, host_out


def kernel(node_feats, w3, w2, w1, U3, U2, U1):
    global LAST_EXEC_NS
    import os
    from concourse.bass_utils import run_bass_kernel_spmd

    if "nc" not in _CACHE:
        _CACHE["nc"] = _build_nc()
    nc = _CACHE["nc"]

    in_maps, host_out = _prep_inputs(node_feats, w3, w2, w1, U3, U2, U1)
    trace = bool(os.environ.get("BASS_TRACE"))
    res = run_bass_kernel_spmd(nc, in_maps, list(range(NCORES)),
                               trace=trace)
    LAST_EXEC_NS = res.exec_time_ns
    _CACHE["last_results"] = res

    out = host_out.copy()
    for r in range(NCORES):
        out += res.results[r]["out"].astype(np.float64)
    return out.astype(np.float32)
